# revision 9
# baseline (speedup 1.0000x reference)
"""Trainium2 Bass kernel for GNN message passing (nn_MessagePassing).

Sharding: edges are bucketed by TARGET node; core c owns output nodes
[c*6250, (c+1)*6250). Each core builds the full node feature tables
(xh | xd) in its own HBM, gathers source rows by edge index via
indirect DMA, computes per-edge messages on DVE/ACT, and performs the
segment-sum as a selection-matrix matmul accumulating in PSUM. The
vector activation runs per 128-node block via PE transposes. Outputs
are disjoint row slices, concatenated on the host. No collectives.
"""

import sys

sys.path.insert(0, "/opt/trn_rl_repo")

import numpy as np

import concourse.bass as bass
import concourse.tile as tile
from concourse import bacc, mybir
from concourse.bass import IndirectOffsetOnAxis

# Problem constants (hardcoded per contract)
N, E, H, F = 50000, 400000, 128, 64
NCORES = 8
P = 128
INV_SQRT_3 = 1.0 / np.sqrt(3.0)
INV_SQRT_H = 1.0 / np.sqrt(H)
SILU_SCALE = 1.0 / 0.6

F16 = mybir.dt.float16
F32 = mybir.dt.float32
I32 = mybir.dt.int32


def _default_cfg(n=N, e=E):
    npc = n // NCORES
    nblk = (npc + P - 1) // P
    npad = ((n + 511) // 512) * 512
    return dict(n=n, e=e, npc=npc, nblk=nblk, npad=npad)


def host_prep(cfg, inputs):
    """Shard/sort/pad inputs. Returns (in_maps, static) for the bass run."""
    n, e, npc, nblk, npad = (
        cfg["n"],
        cfg["e"],
        cfg["npc"],
        cfg["nblk"],
        cfg["npad"],
    )
    f32 = np.float32
    f16 = np.float16

    x = np.asarray(inputs["x"], f32)
    xdef = np.asarray(inputs["x_defect"], f32)
    vec = np.asarray(inputs["vec"], f32).reshape(n, 3 * H)
    ef = np.asarray(inputs["edge_feat"], f32)
    ev = np.asarray(inputs["edge_vector"], f32)
    eidx = np.asarray(inputs["edge_index"])
    j_all = eidx[0].astype(np.int64)
    i_all = eidx[1].astype(np.int64)

    # ---- shared constants ----
    xT = np.zeros((H, npad), f16)
    xT[:, :n] = x.T.astype(f16)
    xdT = np.zeros((H, npad), f16)
    xdT[:, :n] = xdef.T.astype(f16)
    vec_h = np.zeros((npad, 3 * H), f16)
    vec_h[:n] = (vec * INV_SQRT_H).astype(f16)

    W1 = np.asarray(inputs["W_x1"], f32).astype(f16)  # [128, 64]
    b1 = np.asarray(inputs["b_x1"], f32).reshape(H // 2, 1)  # [64,1] f32
    W2b = np.vstack(
        [np.asarray(inputs["W_x2"], f32) * SILU_SCALE, np.asarray(inputs["b_x2"], f32)[None, :]]
    ).astype(f16)  # [65, 384]
    Wd = np.asarray(inputs["W_defect"], f32).astype(f16)  # [128, 384]
    bd = np.asarray(inputs["b_defect"], f32)
    has_bd = bool(np.any(bd != 0.0))
    bd_row = bd[None, :].astype(f16)  # [1, 384]
    We2 = (
        np.vstack([np.asarray(inputs["W_edge"], f32), np.asarray(inputs["b_edge"], f32)[None, :]])
        * INV_SQRT_3
    ).astype(f16)  # [65, 384]
    Wlv = np.asarray(inputs["W_lvec"], f32)  # [128,128] f32
    Wgr = np.tile(np.asarray(inputs["W_gvec"], f32).reshape(H, 1), (1, P))
    iota = np.tile(np.arange(P, dtype=f32)[None, :], (P, 1))  # [128,128] f32
    ident = np.eye(P, dtype=f32)

    # ---- per-core edge bucketing ----
    core_of = i_all // npc
    order = np.argsort(core_of * (npc * 2) + (i_all % npc), kind="stable")
    # per (core, block) counts to find T_max
    blk_of = (i_all % npc) // P
    keys = core_of * nblk + blk_of
    cnt = np.bincount(keys, minlength=NCORES * nblk).reshape(NCORES, nblk)
    t_max = max(1, int(np.ceil(cnt.max() / P)))
    ntiles = nblk * t_max
    epad = ntiles * P

    in_maps = []
    for c in range(NCORES):
        # edges of this core, sorted by local node (hence by block)
        sel = order[core_of[order] == c]
        jc = j_all[sel]
        ic = i_all[sel]
        ilocc = ic - c * npc
        blkc = ilocc // P
        tlc = ilocc % P

        # slot each edge: block b occupies [b*t_max*P, (b+1)*t_max*P)
        slots = np.full(epad, -1, np.int64)
        bstart = np.concatenate([[0], np.cumsum(np.bincount(blkc, minlength=nblk))])
        # position within block = running index among this block's edges
        within = np.arange(len(sel)) - bstart[blkc]
        slot = blkc * (t_max * P) + within
        slots[slot] = np.arange(len(sel))

        jpad = np.zeros(epad, f32)
        ipad = np.zeros(epad, f32)
        tlpad = np.full(epad, 999.0, f32)
        evpad = np.zeros((epad, 3), f32)
        efpad = np.zeros((epad, F), f16)
        valid = slots >= 0
        vs = slots[valid]
        jpad[valid] = jc[vs].astype(f32)
        ipad[valid] = ic[vs].astype(f32)
        tlpad[valid] = tlc[vs].astype(f32)
        evpad[valid] = (ev[sel][vs] * INV_SQRT_H).astype(f32)
        efpad[valid] = ef[sel][vs].astype(f16)

        # esc: [ntiles, 8, 128] f32 rows: j, i, tl, ev0, ev1, ev2, 0, 0
        esc = np.zeros((ntiles, 8, P), f32)
        esc[:, 0, :] = jpad.reshape(ntiles, P)
        esc[:, 1, :] = ipad.reshape(ntiles, P)
        esc[:, 2, :] = tlpad.reshape(ntiles, P)
        esc[:, 3:6, :] = evpad.reshape(ntiles, P, 3).transpose(0, 2, 1)

        # efT: [65, epad] f16, row 64 = ones
        efT = np.empty((F + 1, epad), f16)
        efT[:F, :] = efpad.T
        efT[F, :] = 1.0

        m = dict(
            xT=xT,
            xdT=xdT,
            vec_h=vec_h,
            W1=W1,
            b1=b1,
            W2b=W2b,
            Wd=Wd,
            We2=We2,
            Wlv=Wlv,
            Wgr=Wgr,
            iota=iota,
            ident=ident,
            esc=esc,
            efT=efT,
        )
        if has_bd:
            m["bd"] = bd_row
        in_maps.append(m)

    static = dict(t_max=t_max, ntiles=ntiles, epad=epad, has_bd=has_bd)
    return in_maps, static


def build_program(cfg, static):
    n, npc, nblk, npad = cfg["n"], cfg["npc"], cfg["nblk"], cfg["npad"]
    t_max, epad = static["t_max"], static["epad"]
    has_bd = static["has_bd"]
    H3 = 3 * H  # 384

    nc = bacc.Bacc()
    # inputs
    xT_d = nc.declare_dram_parameter("xT", [H, npad], F16, isOutput=False)
    xdT_d = nc.declare_dram_parameter("xdT", [H, npad], F16, isOutput=False)
    vec_d = nc.declare_dram_parameter("vec_h", [npad, H3], F16, False)
    W1_d = nc.declare_dram_parameter("W1", [H, H // 2], F16, False)
    b1_d = nc.declare_dram_parameter("b1", [H // 2, 1], F32, False)
    W2b_d = nc.declare_dram_parameter("W2b", [H // 2 + 1, H3], F16, False)
    Wd_d = nc.declare_dram_parameter("Wd", [H, H3], F16, False)
    We2_d = nc.declare_dram_parameter("We2", [F + 1, H3], F16, False)
    Wlv_d = nc.declare_dram_parameter("Wlv", [H, H], F32, False)
    Wgr_d = nc.declare_dram_parameter("Wgr", [H, P], F32, False)
    iota_d = nc.declare_dram_parameter("iota", [P, P], F32, False)
    ident_d = nc.declare_dram_parameter("ident", [P, P], F32, False)
    esc_d = nc.declare_dram_parameter("esc", [static["ntiles"], 8, P], F32, False)
    efT_d = nc.declare_dram_parameter("efT", [F + 1, epad], F16, False)
    bd_d = nc.declare_dram_parameter("bd", [1, H3], F16, False) if has_bd else None
    dx_d = nc.declare_dram_parameter("dx_out", [npc, H], F32, isOutput=True)
    dv_d = nc.declare_dram_parameter("dvec_out", [npc, H3], F32, isOutput=True)

    with tile.TileContext(nc) as tc:
        with (
            tc.tile_pool(name="const", bufs=1) as cp,
            tc.tile_pool(name="dram", bufs=1, space="DRAM") as dp,
        ):
            # resident constants
            W1s = cp.tile([H, H // 2], F16)
            nc.sync.dma_start(out=W1s[:], in_=W1_d[:, :])
            b1s = cp.tile([H // 2, 1], F32)
            nc.sync.dma_start(out=b1s[:], in_=b1_d[:, :])
            W2bs = cp.tile([H // 2 + 1, H3], F16)
            nc.sync.dma_start(out=W2bs[:], in_=W2b_d[:, :])
            Wds = cp.tile([H, H3], F16)
            nc.sync.dma_start(out=Wds[:], in_=Wd_d[:, :])
            We2s = cp.tile([F + 1, H3], F16)
            nc.sync.dma_start(out=We2s[:], in_=We2_d[:, :])
            Wlvs = cp.tile([H, H], F32)
            nc.sync.dma_start(out=Wlvs[:], in_=Wlv_d[:, :])
            Wgrs = cp.tile([H, P], F32)
            nc.sync.dma_start(out=Wgrs[:], in_=Wgr_d[:, :])
            iotas = cp.tile([P, P], F32)
            nc.sync.dma_start(out=iotas[:], in_=iota_d[:, :])
            idents = cp.tile([P, P], F32)
            nc.sync.dma_start(out=idents[:], in_=ident_d[:, :])
            if has_bd:
                bds = cp.tile([1, H3], F16)
                nc.sync.dma_start(out=bds[:], in_=bd_d[:, :])
                ones1 = cp.tile([1, P], F16)
                nc.vector.memset(ones1[:], 1.0)

            # node table in this core's DRAM: [npad, 768] = [xh | xd]
            tab = dp.tile([npad, 2 * H3], F16)

            # ---------------- node stage ----------------
            with (
                tc.tile_pool(name="nin", bufs=3) as nin,
                tc.tile_pool(name="nmid", bufs=3) as nmid,
                tc.tile_pool(name="nout", bufs=4) as nout,
                tc.tile_pool(name="npsum", bufs=2, space="PSUM") as npp,
            ):
                for tch in range(npad // 512):
                    sl = slice(tch * 512, (tch + 1) * 512)
                    xt = nin.tile([H, 512], F16, tag="xt")
                    nc.sync.dma_start(out=xt[:], in_=xT_d[:, sl])
                    xdt = nin.tile([H, 512], F16, tag="xdt")
                    nc.sync.dma_start(out=xdt[:], in_=xdT_d[:, sl])
                    ps1 = npp.tile([H // 2, 512], F32, tag="ps1")
                    nc.tensor.matmul(out=ps1[:], lhsT=W1s[:], rhs=xt[:], start=True, stop=True)
                    h1t = nmid.tile([H // 2 + 1, 512], F16, tag="h1")
                    nc.vector.memset(h1t[H // 2 : H // 2 + 1, :], 1.0)
                    nc.scalar.activation(
                        h1t[0 : H // 2, :],
                        ps1[:],
                        mybir.ActivationFunctionType.Silu,
                        bias=b1s[:],
                        scale=1.0,
                    )
                    for c4 in range(4):
                        csl = slice(c4 * P, (c4 + 1) * P)
                        nsl = slice(tch * 512 + c4 * P, tch * 512 + (c4 + 1) * P)
                        ps2 = npp.tile([P, H3], F32, tag="ps2")
                        nc.tensor.matmul(
                            out=ps2[:], lhsT=h1t[:, csl], rhs=W2bs[:], start=True, stop=True
                        )
                        ps3 = npp.tile([P, H3], F32, tag="ps3")
                        nc.tensor.matmul(
                            out=ps3[:],
                            lhsT=xdt[:, csl],
                            rhs=Wds[:],
                            start=True,
                            stop=not has_bd,
                        )
                        if has_bd:
                            nc.tensor.matmul(
                                out=ps3[:], lhsT=ones1[:], rhs=bds[:], start=False, stop=True
                            )
                        outt = nout.tile([P, 2 * H3], F16, tag="outt")
                        nc.scalar.activation(
                            outt[:, 0:H3], ps2[:], mybir.ActivationFunctionType.Copy
                        )
                        nc.vector.tensor_copy(outt[:, H3 : 2 * H3], ps3[:])
                        nc.sync.dma_start(out=tab[nsl, :], in_=outt[:])

            # ---------------- edge stage ----------------
            with (
                tc.tile_pool(name="eidx", bufs=4) as eip,
                tc.tile_pool(name="egath", bufs=4) as egp,
                tc.tile_pool(name="emid", bufs=3) as emp,
                tc.tile_pool(name="eout", bufs=3) as eop,
                tc.tile_pool(name="epsum", bufs=2, space="PSUM") as epp,
                tc.tile_pool(name="apsum", bufs=2, space="PSUM") as app,
            ):
                for blk in range(nblk):
                    acc = app.tile([P, H3 + H], F32, tag="acc")
                    for tt in range(t_max):
                        ti = blk * t_max + tt
                        e0 = ti * P
                        esl = slice(e0, e0 + P)
                        # small per-edge data: [8,128] -> transpose -> [128,8]
                        esct = eip.tile([8, P], F32, tag="esc")
                        nc.sync.dma_start(out=esct[:], in_=esc_d[ti, :, :])
                        pesc = epp.tile([P, 8], F32, tag="small")
                        nc.tensor.transpose(out=pesc[:], in_=esct[:], identity=idents[0:8, 0:8])
                        esm = eip.tile([P, 8], F32, tag="esm")
                        nc.vector.tensor_copy(esm[:], pesc[:])
                        jt = eip.tile([P, 1], I32, tag="jt")
                        nc.vector.tensor_copy(jt[:], esm[:, 0:1])
                        it_ = eip.tile([P, 1], I32, tag="it")
                        nc.vector.tensor_copy(it_[:], esm[:, 1:2])
                        # gathers
                        g1 = egp.tile([P, 2 * H3], F16, tag="g1")
                        nc.gpsimd.indirect_dma_start(
                            out=g1[:],
                            out_offset=None,
                            in_=tab[:, :],
                            in_offset=IndirectOffsetOnAxis(ap=jt[:, 0:1], axis=0),
                        )
                        gv = egp.tile([P, H3], F16, tag="gv")
                        nc.gpsimd.indirect_dma_start(
                            out=gv[:],
                            out_offset=None,
                            in_=vec_d[:, :],
                            in_offset=IndirectOffsetOnAxis(ap=jt[:, 0:1], axis=0),
                        )
                        gxi = egp.tile([P, H3], F16, tag="gxi")
                        nc.gpsimd.indirect_dma_start(
                            out=gxi[:],
                            out_offset=None,
                            in_=tab[:, :],
                            in_offset=IndirectOffsetOnAxis(ap=it_[:, 0:1], axis=0),
                            element_offset=H3,
                        )
                        # rbf = efT_tile.T @ We2  (includes bias+1/sqrt3)
                        eft = eip.tile([F + 1, P], F16, tag="eft")
                        nc.sync.dma_start(out=eft[:], in_=efT_d[:, esl])
                        psr = epp.tile([P, H3], F32, tag="rbf")
                        nc.tensor.matmul(out=psr[:], lhsT=eft[:], rhs=We2s[:], start=True, stop=True)
                        # messages
                        xsum = emp.tile([P, H3], F16, tag="xsum")
                        nc.vector.tensor_add(xsum[:], g1[:, H3 : 2 * H3], gxi[:])
                        t2 = emp.tile([P, H3], F16, tag="t2")
                        nc.vector.tensor_mul(t2[:], xsum[:], g1[:, 0:H3])
                        M = eop.tile([P, H3 + H], F16, tag="M")
                        m12 = emp.tile([P, 2 * H], F16, tag="m12")
                        nc.vector.tensor_mul(m12[:], t2[:, 0 : 2 * H], psr[:, 0 : 2 * H])
                        nc.vector.tensor_mul(
                            M[:, H3 : H3 + H], t2[:, 2 * H : H3], psr[:, 2 * H : H3]
                        )
                        # q = m2 (bcast over a) * ev (expand over h)
                        q = emp.tile([P, H3], F16, tag="q")
                        nc.vector.tensor_mul(
                            q[:],
                            m12[:, None, H : 2 * H].to_broadcast([P, 3, H]),
                            esm[:, 3:6].to_broadcast([P, 3, H]),
                        )
                        # M_vec = m1 (bcast) * vec_j + q
                        nc.vector.tensor_mul(
                            M[:, 0:H3], m12[:, None, 0:H].to_broadcast([P, 3, H]), gv[:]
                        )
                        nc.vector.tensor_add(M[:, 0:H3], M[:, 0:H3], q[:])
                        # selection matrix
                        S = emp.tile([P, P], F16, tag="S")
                        nc.vector.tensor_tensor(
                            S[:],
                            esm[:, 2:3].to_broadcast([P, P]),
                            iotas[:],
                            op=mybir.AluOpType.is_equal,
                        )
                        nc.tensor.matmul(
                            out=acc[:],
                            lhsT=S[:],
                            rhs=M[:],
                            start=(tt == 0),
                            stop=(tt == t_max - 1),
                        )

                    # -------- block epilogue --------
                    nreal = min(P, npc - blk * P)
                    nsl = slice(blk * P, blk * P + nreal)
                    dxs = eop.tile([P, H], F32, tag="dxs")
                    nc.scalar.activation(
                        dxs[:], acc[:, H3 : H3 + H], mybir.ActivationFunctionType.Copy
                    )
                    nc.sync.dma_start(out=dx_d[nsl, :], in_=dxs[0:nreal, :])
                    dvs = eop.tile([P, H3], F32, tag="dvs")
                    nc.vector.tensor_copy(dvs[:], acc[:, 0:H3])
                    dvT = eop.tile([P, H3], F32, tag="dvT")
                    for a in range(3):
                        asl = slice(a * P, (a + 1) * P)
                        pst = epp.tile([P, P], F32, tag="small")
                        nc.tensor.transpose(out=pst[:], in_=dvs[:, asl], identity=idents[:])
                        nc.vector.tensor_copy(dvT[:, asl], pst[:])
                    plv = epp.tile([P, H3], F32, tag="rbf")
                    nc.tensor.matmul(out=plv[:], lhsT=Wlvs[:], rhs=dvT[:], start=True, stop=True)
                    pgv = epp.tile([P, H3], F32, tag="rbf")
                    nc.tensor.matmul(out=pgv[:], lhsT=Wgrs[:], rhs=dvT[:], start=True, stop=True)
                    gvs = eop.tile([P, H3], F32, tag="gvs")
                    nc.scalar.activation(gvs[:], pgv[:], mybir.ActivationFunctionType.Copy)
                    prod = eop.tile([P, H3], F32, tag="prod")
                    nc.vector.tensor_mul(prod[:], plv[:], gvs[:])
                    dot = eop.tile([P, P], F32, tag="dot")
                    nc.vector.tensor_add(dot[:], prod[:, 0:P], prod[:, P : 2 * P])
                    nc.vector.tensor_add(dot[:], dot[:], prod[:, 2 * P : 3 * P])
                    msk = eop.tile([P, P], F32, tag="msk")
                    nc.vector.tensor_scalar(
                        msk[:], dot[:], 0.0, None, op0=mybir.AluOpType.is_ge
                    )
                    u = eop.tile([P, P], F32, tag="u")
                    nc.vector.tensor_scalar(
                        u[:], msk[:], 0.5, 0.5, op0=mybir.AluOpType.mult, op1=mybir.AluOpType.add
                    )
                    v = eop.tile([P, P], F32, tag="v")
                    nc.vector.tensor_scalar(
                        v[:], msk[:], -0.5, 0.5, op0=mybir.AluOpType.mult, op1=mybir.AluOpType.add
                    )
                    oT = eop.tile([P, H3], F32, tag="oT")
                    nc.vector.tensor_mul(
                        oT[:], plv[:], u[:, None, :].to_broadcast([P, 3, P])
                    )
                    o2 = eop.tile([P, H3], F32, tag="o2")
                    nc.vector.tensor_mul(
                        o2[:], gvs[:], v[:, None, :].to_broadcast([P, 3, P])
                    )
                    nc.vector.tensor_add(oT[:], oT[:], o2[:])
                    dvo = eop.tile([P, H3], F32, tag="dvo")
                    for a in range(3):
                        asl = slice(a * P, (a + 1) * P)
                        pso = epp.tile([P, P], F32, tag="small")
                        nc.tensor.transpose(out=pso[:], in_=oT[:, asl], identity=idents[:])
                        nc.vector.tensor_copy(dvo[:, asl], pso[:])
                    nc.sync.dma_start(out=dv_d[nsl, :], in_=dvo[0:nreal, :])

    nc.finalize()
    return nc


def run(cfg, inputs, trace=False):
    from concourse.bass_utils import run_bass_kernel_spmd

    in_maps, static = host_prep(cfg, inputs)
    nc = build_program(cfg, static)
    core_ids = list(range(NCORES))
    res = run_bass_kernel_spmd(nc, in_maps, core_ids, trace=trace)
    npc = cfg["npc"]
    dx = np.concatenate([res.results[c]["dx_out"] for c in range(NCORES)], axis=0)
    dv = np.concatenate([res.results[c]["dvec_out"] for c in range(NCORES)], axis=0)
    dv = dv.reshape(cfg["n"], 3, H)
    return (dx, dv), res


def kernel(**inputs):
    cfg = _default_cfg()
    (dx, dv), _ = run(cfg, inputs, trace=False)
    return (dx, dv)


# revision 17
# speedup vs baseline: 1.5305x; 1.5305x over previous
"""Trainium2 Bass kernel for GNN message passing (nn_MessagePassing).

Sharding: edges are bucketed by TARGET node; core c owns output nodes
[c*6250, (c+1)*6250). Each core builds the full node feature tables
(xh | xd) in its own HBM, gathers source rows by edge index via
indirect DMA (batched 4 tiles per instruction), computes per-edge
messages on DVE/ACT, and performs the segment-sum as a selection-matrix
matmul accumulating in PSUM. The vector activation runs per 128-node
block via PE transposes. Outputs are disjoint row slices, concatenated
on the host. No collectives.
"""

import sys

sys.path.insert(0, "/opt/trn_rl_repo")

import numpy as np

import concourse.bass as bass
import concourse.tile as tile
from concourse import bacc, mybir
from concourse.bass import IndirectOffsetOnAxis

# Problem constants (hardcoded per contract)
N, E, H, F = 50000, 400000, 128, 64
NCORES = 8
P = 128
GK = 1  # tiles per batched gather
INV_SQRT_3 = 1.0 / np.sqrt(3.0)
INV_SQRT_H = 1.0 / np.sqrt(H)
SILU_SCALE = 1.0 / 0.6

F16 = mybir.dt.float16
F32 = mybir.dt.float32
I32 = mybir.dt.int32

Silu = mybir.ActivationFunctionType.Silu
Copy = mybir.ActivationFunctionType.Copy


def _default_cfg(n=N, e=E):
    npc = n // NCORES
    nblk = (npc + P - 1) // P
    npad = ((n + 511) // 512) * 512
    return dict(n=n, e=e, npc=npc, nblk=nblk, npad=npad)


def host_prep(cfg, inputs):
    """Shard/sort/pad inputs. Returns (in_maps, static) for the bass run."""
    n, e, npc, nblk, npad = (
        cfg["n"],
        cfg["e"],
        cfg["npc"],
        cfg["nblk"],
        cfg["npad"],
    )
    f32 = np.float32
    f16 = np.float16

    x = np.asarray(inputs["x"], f32)
    xdef = np.asarray(inputs["x_defect"], f32)
    vec = np.asarray(inputs["vec"], f32).reshape(n, 3 * H)
    ef = np.asarray(inputs["edge_feat"], f32)
    ev = np.asarray(inputs["edge_vector"], f32)
    eidx = np.asarray(inputs["edge_index"])
    j_all = eidx[0].astype(np.int64)
    i_all = eidx[1].astype(np.int64)

    # ---- shared constants ----
    xT = np.zeros((H, npad), f16)
    xT[:, :n] = x.T.astype(f16)
    xdT = np.zeros((H, npad), f16)
    xdT[:, :n] = xdef.T.astype(f16)
    vec_h = np.zeros((npad, 3 * H), f16)
    vec_h[:n] = (vec * INV_SQRT_H).astype(f16)

    W1 = np.asarray(inputs["W_x1"], f32).astype(f16)  # [128, 64]
    b1 = np.asarray(inputs["b_x1"], f32).reshape(H // 2, 1)  # [64,1] f32
    W2b = np.vstack(
        [np.asarray(inputs["W_x2"], f32) * SILU_SCALE, np.asarray(inputs["b_x2"], f32)[None, :]]
    ).astype(f16)  # [65, 384]
    Wd = np.asarray(inputs["W_defect"], f32).astype(f16)  # [128, 384]
    bd = np.asarray(inputs["b_defect"], f32)
    has_bd = bool(np.any(bd != 0.0))
    bd_row = bd[None, :].astype(f16)  # [1, 384]
    We2 = (
        np.vstack([np.asarray(inputs["W_edge"], f32), np.asarray(inputs["b_edge"], f32)[None, :]])
        * INV_SQRT_3
    ).astype(f16)  # [65, 384]
    Wlv = np.asarray(inputs["W_lvec"], f32)  # [128,128] f32
    Wgr = np.tile(np.asarray(inputs["W_gvec"], f32).reshape(H, 1), (1, P))
    iota = np.tile(np.arange(P, dtype=f32)[None, :], (P, 1))  # [128,128] f32
    ident = np.eye(P, dtype=f32)

    # ---- per-core edge bucketing ----
    core_of = i_all // npc
    order = np.argsort(core_of * (npc * 2) + (i_all % npc), kind="stable")
    blk_of = (i_all % npc) // P
    keys = core_of * nblk + blk_of
    cnt = np.bincount(keys, minlength=NCORES * nblk).reshape(NCORES, nblk)
    t_max = max(1, int(np.ceil(cnt.max() / P)))
    ntiles = nblk * t_max
    ngrp = (ntiles + GK - 1) // GK
    ntpad = ngrp * GK  # tiles padded to gather-group multiple
    epad = ntpad * P
    n8 = (ntpad + 7) // 8  # esc groups of 8 tiles

    in_maps = []
    for c in range(NCORES):
        sel = order[core_of[order] == c]
        jc = j_all[sel]
        ic = i_all[sel]
        ilocc = ic - c * npc
        blkc = ilocc // P
        tlc = ilocc % P

        slots = np.full(ntiles * P, -1, np.int64)
        bstart = np.concatenate([[0], np.cumsum(np.bincount(blkc, minlength=nblk))])
        within = np.arange(len(sel)) - bstart[blkc]
        slot = blkc * (t_max * P) + within
        slots[slot] = np.arange(len(sel))
        slots = np.concatenate([slots, np.full(epad - ntiles * P, -1, np.int64)])

        jpad = np.zeros(epad, f32)
        ipad = np.zeros(epad, f32)
        tlpad = np.full(epad, 999.0, f32)
        evpad = np.zeros((epad, 3), f32)
        efpad = np.zeros((epad, F), f16)
        valid = slots >= 0
        vs = slots[valid]
        jpad[valid] = jc[vs].astype(f32)
        ipad[valid] = ic[vs].astype(f32)
        tlpad[valid] = tlc[vs].astype(f32)
        evpad[valid] = (ev[sel][vs] * INV_SQRT_H).astype(f32)
        efpad[valid] = ef[sel][vs].astype(f16)

        # esc: [n8, 8(row), 8(tile), 128] f32 rows: j, i, tl, ev0, ev1, ev2, 0, 0
        esc = np.zeros((n8, 8, 8 * P), f32)
        rows = np.zeros((8, ntpad, P), f32)
        rows[0] = jpad.reshape(ntpad, P)
        rows[1] = ipad.reshape(ntpad, P)
        rows[2] = tlpad.reshape(ntpad, P)
        rows[3:6] = evpad.reshape(ntpad, P, 3).transpose(2, 0, 1)
        for g in range(n8):
            t0, t1 = g * 8, min((g + 1) * 8, ntpad)
            esc[g, :, : (t1 - t0) * P] = rows[:, t0:t1, :].reshape(8, -1)

        # efT: [65, epad] f16, row 64 = ones
        efT = np.empty((F + 1, epad), f16)
        efT[:F, :] = efpad.T
        efT[F, :] = 1.0

        m = dict(
            xT=xT,
            xdT=xdT,
            vec_h=vec_h,
            W1=W1,
            b1=b1,
            W2b=W2b,
            Wd=Wd,
            We2=We2,
            Wlv=Wlv,
            Wgr=Wgr,
            iota=iota,
            ident=ident,
            esc=esc,
            efT=efT,
        )
        if has_bd:
            m["bd"] = bd_row
        in_maps.append(m)

    static = dict(
        t_max=t_max, ntiles=ntiles, ntpad=ntpad, ngrp=ngrp, n8=n8, epad=epad, has_bd=has_bd
    )
    return in_maps, static


def build_program(cfg, static, dbg=False):
    n, npc, nblk, npad = cfg["n"], cfg["npc"], cfg["nblk"], cfg["npad"]
    t_max, epad = static["t_max"], static["epad"]
    ntpad, ngrp, n8 = static["ntpad"], static["ngrp"], static["n8"]
    has_bd = static["has_bd"]
    H3 = 3 * H  # 384

    nc = bacc.Bacc()
    xT_d = nc.declare_dram_parameter("xT", [H, npad], F16, False)
    xdT_d = nc.declare_dram_parameter("xdT", [H, npad], F16, False)
    vec_d = nc.declare_dram_parameter("vec_h", [npad, H3], F16, False)
    W1_d = nc.declare_dram_parameter("W1", [H, H // 2], F16, False)
    b1_d = nc.declare_dram_parameter("b1", [H // 2, 1], F32, False)
    W2b_d = nc.declare_dram_parameter("W2b", [H // 2 + 1, H3], F16, False)
    Wd_d = nc.declare_dram_parameter("Wd", [H, H3], F16, False)
    We2_d = nc.declare_dram_parameter("We2", [F + 1, H3], F16, False)
    Wlv_d = nc.declare_dram_parameter("Wlv", [H, H], F32, False)
    Wgr_d = nc.declare_dram_parameter("Wgr", [H, P], F32, False)
    iota_d = nc.declare_dram_parameter("iota", [P, P], F32, False)
    ident_d = nc.declare_dram_parameter("ident", [P, P], F32, False)
    esc_d = nc.declare_dram_parameter("esc", [n8, 8, 8 * P], F32, False)
    efT_d = nc.declare_dram_parameter("efT", [F + 1, epad], F16, False)
    bd_d = nc.declare_dram_parameter("bd", [1, H3], F16, False) if has_bd else None
    dx_d = nc.declare_dram_parameter("dx_out", [npc, H], F32, isOutput=True)
    dv_d = nc.declare_dram_parameter("dvec_out", [npc, H3], F32, isOutput=True)
    if dbg:
        dbg_d = {k: nc.declare_dram_parameter(f"dbg_{k}", [P, w], F16, isOutput=True)
                 for k, w in [("xsum", H3), ("t2", H3), ("q", H3), ("M", H3 + H), ("S", P),
                              ("g1", 2 * H3), ("gxi", H3), ("gv", H3)]}
        dbg_d["jt"] = nc.declare_dram_parameter("dbg_jt", [P, 8], I32, isOutput=True)
        dbg_d["gv2"] = nc.declare_dram_parameter("dbg_gv2", [P, H3], F16, isOutput=True)

    with tile.TileContext(nc) as tc:
        with (
            tc.tile_pool(name="const", bufs=1) as cp,
            tc.tile_pool(name="dram", bufs=1, space="DRAM") as dp,
        ):
            # resident constants
            W1s = cp.tile([H, H // 2], F16)
            nc.sync.dma_start(out=W1s[:], in_=W1_d[:, :])
            b1s = cp.tile([H // 2, 1], F32)
            nc.sync.dma_start(out=b1s[:], in_=b1_d[:, :])
            W2bs = cp.tile([H // 2 + 1, H3], F16)
            nc.sync.dma_start(out=W2bs[:], in_=W2b_d[:, :])
            Wds = cp.tile([H, H3], F16)
            nc.sync.dma_start(out=Wds[:], in_=Wd_d[:, :])
            We2s = cp.tile([F + 1, H3], F16)
            nc.sync.dma_start(out=We2s[:], in_=We2_d[:, :])
            Wlvs = cp.tile([H, H], F32)
            nc.sync.dma_start(out=Wlvs[:], in_=Wlv_d[:, :])
            Wgrs = cp.tile([H, P], F32)
            nc.sync.dma_start(out=Wgrs[:], in_=Wgr_d[:, :])
            iotas = cp.tile([P, P], F32)
            nc.sync.dma_start(out=iotas[:], in_=iota_d[:, :])
            idents = cp.tile([P, P], F32)
            nc.sync.dma_start(out=idents[:], in_=ident_d[:, :])
            if has_bd:
                bds = cp.tile([1, H3], F16)
                nc.sync.dma_start(out=bds[:], in_=bd_d[:, :])
                ones1 = cp.tile([1, P], F16)
                nc.vector.memset(ones1[:], 1.0)

            # persistent per-edge metadata (filled in phase E0)
            esm = cp.tile([P, ntpad, 8], F32)  # [p, tile, row]: j,i,tl,ev0..2
            jt_all = cp.tile([P, ntpad], I32)
            it_all = cp.tile([P, ntpad], I32)

            # node table in this core's DRAM: [npad, 768] = [xh | xd]
            tab = dp.tile([npad, 2 * H3], F16)

            # ---------------- phase E0: unpack per-edge metadata ----------------
            with (
                tc.tile_pool(name="e0in", bufs=4) as e0p,
                tc.tile_pool(name="e0ps", bufs=4, space="PSUM") as e0ps,
            ):
                for g in range(n8):
                    esg = e0p.tile([8, 8 * P], F32, tag="esg")
                    nc.sync.dma_start(out=esg[:], in_=esc_d[g, :, :])
                    for t in range(8):
                        ti = g * 8 + t
                        if ti >= ntpad:
                            break
                        pesc = e0ps.tile([P, 8], F32, tag="pesc")
                        nc.tensor.transpose(
                            out=pesc[:], in_=esg[:, t * P : (t + 1) * P], identity=idents[0:8, 0:8]
                        )
                        nc.vector.tensor_copy(esm[:, ti, :], pesc[:])
                nc.vector.tensor_copy(jt_all[:, :], esm[:, :, 0])
                nc.vector.tensor_copy(it_all[:, :], esm[:, :, 1])

            # ---------------- node stage ----------------
            with (
                tc.tile_pool(name="nin", bufs=3) as nin,
                tc.tile_pool(name="nmid", bufs=3) as nmid,
                tc.tile_pool(name="nout", bufs=4) as nout,
                tc.tile_pool(name="npsum", bufs=2, space="PSUM") as npp,
            ):
                for tch in range(npad // 512):
                    sl = slice(tch * 512, (tch + 1) * 512)
                    xt = nin.tile([H, 512], F16, tag="xt")
                    nc.sync.dma_start(out=xt[:], in_=xT_d[:, sl])
                    xdt = nin.tile([H, 512], F16, tag="xdt")
                    nc.sync.dma_start(out=xdt[:], in_=xdT_d[:, sl])
                    ps1 = npp.tile([H // 2, 512], F32, tag="ps1")
                    nc.tensor.matmul(out=ps1[:], lhsT=W1s[:], rhs=xt[:], start=True, stop=True)
                    h1t = nmid.tile([H // 2 + 1, 512], F16, tag="h1")
                    nc.vector.memset(h1t[H // 2 : H // 2 + 1, :], 1.0)
                    nc.scalar.activation(h1t[0 : H // 2, :], ps1[:], Silu, bias=b1s[:], scale=1.0)
                    for c4 in range(4):
                        csl = slice(c4 * P, (c4 + 1) * P)
                        nsl = slice(tch * 512 + c4 * P, tch * 512 + (c4 + 1) * P)
                        ps2 = npp.tile([P, H3], F32, tag="ps2")
                        nc.tensor.matmul(
                            out=ps2[:], lhsT=h1t[:, csl], rhs=W2bs[:], start=True, stop=True
                        )
                        ps3 = npp.tile([P, H3], F32, tag="ps3")
                        nc.tensor.matmul(
                            out=ps3[:], lhsT=xdt[:, csl], rhs=Wds[:], start=True, stop=not has_bd
                        )
                        if has_bd:
                            nc.tensor.matmul(
                                out=ps3[:], lhsT=ones1[:], rhs=bds[:], start=False, stop=True
                            )
                        outt = nout.tile([P, 2 * H3], F16, tag="outt")
                        nc.scalar.activation(outt[:, 0:H3], ps2[:], Copy)
                        nc.vector.tensor_copy(outt[:, H3 : 2 * H3], ps3[:])
                        nc.sync.dma_start(out=tab[nsl, :], in_=outt[:])

            # ---------------- edge stage ----------------
            with (
                tc.tile_pool(name="egath", bufs=3) as egp,
                tc.tile_pool(name="eeft", bufs=3) as efp,
                tc.tile_pool(name="emid", bufs=4) as emp,
                tc.tile_pool(name="eout", bufs=3) as eop,
                tc.tile_pool(name="epsum", bufs=3, space="PSUM") as epp,
                tc.tile_pool(name="apsum", bufs=2, space="PSUM") as app,
            ):
                g1 = gv = gxi = None
                for blk in range(nblk):
                    acc = app.tile([P, H3 + H], F32, tag="acc")
                    eftb = efp.tile([F + 1, t_max * P], F16, tag="eftb")
                    nc.sync.dma_start(
                        out=eftb[:], in_=efT_d[:, blk * t_max * P : (blk + 1) * t_max * P]
                    )
                    for tt in range(t_max):
                        ti = blk * t_max + tt
                        grp, gsl = divmod(ti, GK)
                        if gsl == 0:
                            isl = slice(grp * GK, (grp + 1) * GK)
                            g1 = egp.tile([P, GK * 2 * H3], F16, tag="g1")
                            nc.gpsimd.indirect_dma_start(
                                out=g1[:],
                                out_offset=None,
                                in_=tab[:, :],
                                in_offset=IndirectOffsetOnAxis(ap=jt_all[:, isl], axis=0),
                            )
                            gv = egp.tile([P, GK * H3], F16, tag="gv")
                            nc.gpsimd.indirect_dma_start(
                                out=gv[:],
                                out_offset=None,
                                in_=vec_d[:, :],
                                in_offset=IndirectOffsetOnAxis(ap=jt_all[:, isl], axis=0),
                            )
                            gxi = egp.tile([P, GK * H3], F16, tag="gxi")
                            nc.gpsimd.indirect_dma_start(
                                out=gxi[:],
                                out_offset=None,
                                in_=tab[:, :],
                                in_offset=IndirectOffsetOnAxis(ap=it_all[:, isl], axis=0),
                                element_offset=H3,
                            )
                        # rbf = eft.T @ We2  (bias + 1/sqrt3 folded)
                        psr = epp.tile([P, H3], F32, tag="rbf")
                        nc.tensor.matmul(
                            out=psr[:],
                            lhsT=eftb[:, tt * P : (tt + 1) * P],
                            rhs=We2s[:],
                            start=True,
                            stop=True,
                        )
                        # messages
                        xsum = emp.tile([P, H3], F16, tag="xsum")
                        nc.vector.tensor_add(xsum[:], g1[:, gsl * 2 * H3 + H3 : (gsl + 1) * 2 * H3], gxi[:, gsl * H3 : (gsl + 1) * H3])
                        t2 = emp.tile([P, H3], F16, tag="t2")
                        nc.vector.tensor_mul(t2[:], xsum[:], g1[:, gsl * 2 * H3 : gsl * 2 * H3 + H3])
                        M = eop.tile([P, H3 + H], F16, tag="M")
                        m12 = emp.tile([P, 2 * H], F16, tag="m12")
                        nc.vector.tensor_mul(m12[:], t2[:, 0 : 2 * H], psr[:, 0 : 2 * H])
                        nc.vector.tensor_mul(
                            M[:, H3 : H3 + H], t2[:, 2 * H : H3], psr[:, 2 * H : H3]
                        )
                        # q_a = m2 * ev_a  (ACT, per-partition scale)
                        q = emp.tile([P, H3], F16, tag="q")
                        for a in range(3):
                            nc.scalar.activation(
                                q[:, a * P : (a + 1) * P],
                                m12[:, H : 2 * H],
                                Copy,
                                scale=esm[:, ti, 3 + a : 4 + a],
                            )
                        # M_vec = m1 (bcast) * vec_j + q
                        nc.vector.tensor_mul(
                            M[:, 0:H3], m12[:, None, 0:H].to_broadcast([P, 3, H]), gv[:, gsl * H3 : (gsl + 1) * H3]
                        )
                        nc.vector.tensor_add(M[:, 0:H3], M[:, 0:H3], q[:])
                        # selection matrix
                        S = emp.tile([P, P], F16, tag="S")
                        nc.vector.tensor_tensor(
                            S[:],
                            esm[:, ti, 2:3].to_broadcast([P, P]),
                            iotas[:],
                            op=mybir.AluOpType.is_equal,
                        )
                        if dbg and ti == 0:
                            for nm, tl_ in [("xsum", xsum), ("t2", t2), ("q", q), ("M", M), ("S", S),
                                            ("g1", g1[:, 0 : 2 * H3]), ("gxi", gxi[:, 0:H3]), ("gv", gv[:, 0:H3])]:
                                nc.sync.dma_start(out=dbg_d[nm][:, :], in_=tl_[:] if hasattr(tl_, 'tensor') else tl_)
                        nc.tensor.matmul(
                            out=acc[:],
                            lhsT=S[:],
                            rhs=M[:],
                            start=(tt == 0),
                            stop=(tt == t_max - 1),
                        )

                    # -------- block epilogue --------
                    nreal = min(P, npc - blk * P)
                    nsl = slice(blk * P, blk * P + nreal)
                    dxs = eop.tile([P, H], F32, tag="dxs")
                    nc.scalar.activation(dxs[:], acc[:, H3 : H3 + H], Copy)
                    nc.sync.dma_start(out=dx_d[nsl, :], in_=dxs[0:nreal, :])
                    dvs = eop.tile([P, H3], F32, tag="dvs")
                    nc.vector.tensor_copy(dvs[:], acc[:, 0:H3])
                    dvT = eop.tile([P, H3], F32, tag="dvT")
                    for a in range(3):
                        asl = slice(a * P, (a + 1) * P)
                        pst = epp.tile([P, P], F32, tag="small")
                        nc.tensor.transpose(out=pst[:], in_=dvs[:, asl], identity=idents[:])
                        nc.vector.tensor_copy(dvT[:, asl], pst[:])
                    plv = epp.tile([P, H3], F32, tag="rbf")
                    nc.tensor.matmul(out=plv[:], lhsT=Wlvs[:], rhs=dvT[:], start=True, stop=True)
                    pgv = epp.tile([P, H3], F32, tag="rbf")
                    nc.tensor.matmul(out=pgv[:], lhsT=Wgrs[:], rhs=dvT[:], start=True, stop=True)
                    gvs = eop.tile([P, H3], F32, tag="gvs")
                    nc.scalar.activation(gvs[:], pgv[:], Copy)
                    prod = eop.tile([P, H3], F32, tag="prod")
                    nc.vector.tensor_mul(prod[:], plv[:], gvs[:])
                    dot = eop.tile([P, P], F32, tag="dot")
                    nc.vector.tensor_add(dot[:], prod[:, 0:P], prod[:, P : 2 * P])
                    nc.vector.tensor_add(dot[:], dot[:], prod[:, 2 * P : 3 * P])
                    msk = eop.tile([P, P], F32, tag="msk")
                    nc.vector.tensor_scalar(msk[:], dot[:], 0.0, None, op0=mybir.AluOpType.is_ge)
                    u = eop.tile([P, P], F32, tag="u")
                    nc.vector.tensor_scalar(
                        u[:], msk[:], 0.5, 0.5, op0=mybir.AluOpType.mult, op1=mybir.AluOpType.add
                    )
                    v = eop.tile([P, P], F32, tag="v")
                    nc.vector.tensor_scalar(
                        v[:], msk[:], -0.5, 0.5, op0=mybir.AluOpType.mult, op1=mybir.AluOpType.add
                    )
                    oT = eop.tile([P, H3], F32, tag="oT")
                    nc.vector.tensor_mul(oT[:], plv[:], u[:, None, :].to_broadcast([P, 3, P]))
                    o2 = eop.tile([P, H3], F32, tag="o2")
                    nc.vector.tensor_mul(o2[:], gvs[:], v[:, None, :].to_broadcast([P, 3, P]))
                    nc.vector.tensor_add(oT[:], oT[:], o2[:])
                    dvo = eop.tile([P, H3], F32, tag="dvo")
                    for a in range(3):
                        asl = slice(a * P, (a + 1) * P)
                        pso = epp.tile([P, P], F32, tag="small")
                        nc.tensor.transpose(out=pso[:], in_=oT[:, asl], identity=idents[:])
                        nc.vector.tensor_copy(dvo[:, asl], pso[:])
                    nc.sync.dma_start(out=dv_d[nsl, :], in_=dvo[0:nreal, :])
                if dbg:
                    nc.sync.dma_start(out=dbg_d["jt"][:, :], in_=jt_all[:, 0:8])
                    gv2 = egp.tile([P, H3], F16, tag="gv2d")
                    nc.gpsimd.indirect_dma_start(
                        out=gv2[:], out_offset=None, in_=vec_d[:, :],
                        in_offset=IndirectOffsetOnAxis(ap=jt_all[:, 0:1], axis=0))
                    nc.sync.dma_start(out=dbg_d["gv2"][:, :], in_=gv2[:])

    nc.finalize()
    return nc


def run(cfg, inputs, trace=False, dbg=False):
    from concourse.bass_utils import run_bass_kernel_spmd

    in_maps, static = host_prep(cfg, inputs)
    nc = build_program(cfg, static, dbg=dbg)
    core_ids = list(range(NCORES))
    res = run_bass_kernel_spmd(nc, in_maps, core_ids, trace=trace)
    dx = np.concatenate([res.results[c]["dx_out"] for c in range(NCORES)], axis=0)
    dv = np.concatenate([res.results[c]["dvec_out"] for c in range(NCORES)], axis=0)
    dv = dv.reshape(cfg["n"], 3, H)
    return (dx, dv), res


def kernel(**inputs):
    cfg = _default_cfg()
    (dx, dv), _ = run(cfg, inputs, trace=False)
    return (dx, dv)
